# revision 2
# baseline (speedup 1.0000x reference)
"""Positional-encoding add for Trainium2 (8 NeuronCores).

out[b, s, d] = x[b, s, d] + pe[s, d],  x: [8, 4096, 1024] f32.

Sharding: split the seq axis (4096) into 8 chunks of 512 — core c gets
x[:, c*512:(c+1)*512, :] plus its pe slice, so per-core HBM traffic is
minimal (batch sharding would re-read the full pe table on every core).

Precision: the kernel streams x/pe/out through the device in float16.
The correctness budget is rel_err < 2e-2; fp16 quantization of x, pe
and the sum contributes ~4e-4 norm-relative error (fp16 has 10 mantissa
bits; values are O(1) randn + a bounded table), a ~50x margin. This
halves HBM/DMA traffic — the binding resource for this memory-bound
problem — vs the f32 pipeline. The host converts f32 -> f16 before
dispatch and upcasts the result back to f32.

Device layout: the flat [8*512, 1024] fp16 shard is viewed as
[1024, 4096]. 512 consecutive flat rows are exactly one batch, so every
[128, 4096] tile of the view adds the SAME [128, 4096] view of the pe
slice (partition p of the view holds seq rows 4p..4p+3 in both x and
pe). pe loads into SBUF once; 8 1-MiB x tiles stream through
tensor_add (fp16 packed operands hit the DVE 2x mode).
"""

import numpy as np

import concourse.bass as bass
import concourse.mybir as mybir
from concourse.bass_utils import run_bass_kernel_spmd

B, S, D = 8, 4096, 1024
NCORES = 8
S_SH = S // NCORES            # 512 seq positions per core
P = 128                       # SBUF partitions
W = 4096                      # free width of the device view
RV = (B * S_SH * D) // W      # 1024 device-view rows per core
NT = RV // P                  # 8 tiles per core

_CACHE = {}


def _positional_table() -> np.ndarray:
    # Same jnp (XLA CPU) fp32 ops as the reference, then rounded to fp16.
    import jax
    import jax.numpy as jnp

    cpu = jax.devices("cpu")[0]
    with jax.default_device(cpu):
        pos = jnp.arange(S, dtype=jnp.float32)[:, None]
        even = jnp.arange(0, D, 2, dtype=jnp.float32) / D
        odd = jnp.arange(1, D, 2, dtype=jnp.float32) / D
        sin_part = jnp.sin(pos / jnp.power(10000.0, even))
        cos_part = jnp.cos(pos / jnp.power(10000.0, odd))
        pe = jnp.concatenate([sin_part, cos_part], axis=-1)[:, :D]
        return np.asarray(pe).astype(np.float16)


def _build_program():
    # Raw Bass (no TileContext): this container's walrus permits only ONE
    # embedded sync wait per instruction, which Tile's scheduler (and its
    # mandatory tail Drain) exceeds. Explicit wait_ge ops are standalone
    # single-sem instructions and compile fine.
    from contextlib import ExitStack

    nc = bass.Bass("TRN2")
    x = nc.declare_dram_parameter("x", [RV, W], mybir.dt.float16, isOutput=False)
    pe = nc.declare_dram_parameter("pe", [P, W], mybir.dt.float16, isOutput=False)
    out = nc.declare_dram_parameter("out", [RV, W], mybir.dt.float16, isOutput=True)

    with ExitStack() as st:
        pe_sb = st.enter_context(nc.sbuf_tensor("pe_sb", [P, W], mybir.dt.float16))
        tiles = [
            st.enter_context(nc.sbuf_tensor(f"t{i}", [P, W], mybir.dt.float16))
            for i in range(NT)
        ]
        pe_sem = st.enter_context(nc.semaphore("pe_sem"))
        x_sems = [st.enter_context(nc.semaphore(f"x_sem{i}")) for i in range(NT)]
        add_sem = st.enter_context(nc.semaphore("add_sem"))
        done_sem = st.enter_context(nc.semaphore("done_sem"))
        block = st.enter_context(nc.Block())

        @block.sync
        def _(sync):
            # pe split into NT column chunks so the one-time table load
            # spreads across all DMA queues instead of doubling one
            # queue's traffic. All chunks bump one sem: single-wait consume.
            pc = W // NT
            for j in range(NT):
                sync.dma_start(
                    out=pe_sb[:, j * pc:(j + 1) * pc],
                    in_=pe[:, j * pc:(j + 1) * pc],
                ).then_inc(pe_sem, 16)
            for i in range(NT):
                sync.dma_start(
                    out=tiles[i][:], in_=x[i * P:(i + 1) * P, :]
                ).then_inc(x_sems[i], 16)

        @block.vector
        def _(vector):
            vector.wait_ge(pe_sem, 16 * NT)
            for i in range(NT):
                vector.wait_ge(x_sems[i], 16)
                nc.vector.tensor_add(
                    out=tiles[i][:], in0=tiles[i][:], in1=pe_sb[:]
                ).then_inc(add_sem, 1)

        @block.gpsimd
        def _(gpsimd):
            for i in range(NT):
                gpsimd.wait_ge(add_sem, i + 1)
                gpsimd.dma_start(
                    out=out[i * P:(i + 1) * P, :], in_=tiles[i][:]
                ).then_inc(done_sem, 16)
            gpsimd.wait_ge(done_sem, 16 * NT)
    return nc


def _get_program():
    if "nc" not in _CACHE:
        _CACHE["nc"] = _build_program()
        _CACHE["pe"] = _positional_table()
    return _CACHE["nc"], _CACHE["pe"]


def kernel(x: np.ndarray, _trace: bool = False):
    nc, pe = _get_program()
    x = np.asarray(x)
    in_maps = []
    for c in range(NCORES):
        xs = (
            np.ascontiguousarray(x[:, c * S_SH:(c + 1) * S_SH, :])
            .astype(np.float16)
            .reshape(RV, W)
        )
        ps = np.ascontiguousarray(pe[c * S_SH:(c + 1) * S_SH, :]).reshape(P, W)
        in_maps.append({"x": xs, "pe": ps})
    res = run_bass_kernel_spmd(nc, in_maps, list(range(NCORES)), trace=_trace)
    out = np.empty((B, S, D), dtype=np.float32)
    for c in range(NCORES):
        out[:, c * S_SH:(c + 1) * S_SH, :] = (
            res.results[c]["out"].astype(np.float32).reshape(B, S_SH, D)
        )
    if _trace:
        return out, res
    return out


# revision 4
# speedup vs baseline: 1.9322x; 1.9322x over previous
"""Positional-encoding add for Trainium2 (8 NeuronCores).

out[b, s, d] = x[b, s, d] + pe[s, d],  x: [8, 4096, 1024] f32.

Sharding: split the seq axis (4096) into 8 chunks of 512 — core c gets
x[:, c*512:(c+1)*512, :] plus its pe slice, so per-core HBM traffic is
minimal (batch sharding would re-read the full pe table on every core).

Precision: the kernel streams x/pe/out through the device in float16.
The correctness budget is rel_err < 2e-2; fp16 quantization of x, pe
and the sum contributes ~4e-4 norm-relative error (fp16 has 10 mantissa
bits; values are O(1) randn + a bounded table), a ~50x margin. This
halves HBM/DMA traffic — the binding resource for this memory-bound
problem — vs the f32 pipeline. The host converts f32 -> f16 before
dispatch and upcasts the result back to f32.

Device layout: the flat [8*512, 1024] fp16 shard is viewed as
[1024, 4096]. 512 consecutive flat rows are exactly one batch, so every
[128, 4096] tile of the view adds the SAME [128, 4096] view of the pe
slice (partition p of the view holds seq rows 4p..4p+3 in both x and
pe). pe loads into SBUF once; 8 1-MiB x tiles stream through
tensor_add (fp16 packed operands hit the DVE 2x mode).
"""

import numpy as np

import concourse.bass as bass
import concourse.mybir as mybir
from concourse.bass_utils import run_bass_kernel_spmd

B, S, D = 8, 4096, 1024
NCORES = 8
S_SH = S // NCORES            # 512 seq positions per core
P = 128                       # SBUF partitions
W = 4096                      # free width of the device view
RV = (B * S_SH * D) // W      # 1024 device-view rows per core
NT = RV // P                  # 8 tiles per core

_CACHE = {}


def _positional_table() -> np.ndarray:
    # Same jnp (XLA CPU) fp32 ops as the reference, then rounded to fp16.
    import jax
    import jax.numpy as jnp

    cpu = jax.devices("cpu")[0]
    with jax.default_device(cpu):
        pos = jnp.arange(S, dtype=jnp.float32)[:, None]
        even = jnp.arange(0, D, 2, dtype=jnp.float32) / D
        odd = jnp.arange(1, D, 2, dtype=jnp.float32) / D
        sin_part = jnp.sin(pos / jnp.power(10000.0, even))
        cos_part = jnp.cos(pos / jnp.power(10000.0, odd))
        pe = jnp.concatenate([sin_part, cos_part], axis=-1)[:, :D]
        return np.asarray(pe).astype(np.float16)


def _build_program():
    # Raw Bass (no TileContext): this container's walrus permits only ONE
    # embedded sync wait per instruction, which Tile's scheduler (and its
    # mandatory tail Drain) exceeds. Explicit wait_ge ops are standalone
    # single-sem instructions and compile fine.
    from contextlib import ExitStack

    nc = bass.Bass("TRN2")
    x = nc.declare_dram_parameter("x", [RV, W], mybir.dt.float16, isOutput=False)
    pe = nc.declare_dram_parameter("pe", [P, W], mybir.dt.float16, isOutput=False)
    out = nc.declare_dram_parameter("out", [RV, W], mybir.dt.float16, isOutput=True)

    with ExitStack() as st:
        pe_sb = st.enter_context(nc.sbuf_tensor("pe_sb", [P, W], mybir.dt.float16))
        tiles = [
            st.enter_context(nc.sbuf_tensor(f"t{i}", [P, W], mybir.dt.float16))
            for i in range(NT)
        ]
        pe_sem = st.enter_context(nc.semaphore("pe_sem"))
        x_sems = [st.enter_context(nc.semaphore(f"x_sem{i}")) for i in range(NT)]
        add_sem = st.enter_context(nc.semaphore("add_sem"))
        done_sem = st.enter_context(nc.semaphore("done_sem"))
        block = st.enter_context(nc.Block())

        @block.sync
        def _(sync):
            # pe in ONE DMA: each HWDGE instruction serializes ~625ns on the
            # shared HWDGE device, so small chunks starve the DMA engines
            # (625ns issue vs 364ns transfer); one 8KiB/partition descriptor
            # set keeps them saturated.
            sync.dma_start(out=pe_sb[:], in_=pe[:]).then_inc(pe_sem, 16)
            for i in range(NT):
                sync.dma_start(
                    out=tiles[i][:], in_=x[i * P:(i + 1) * P, :]
                ).then_inc(x_sems[i], 16)

        @block.vector
        def _(vector):
            vector.wait_ge(pe_sem, 16)
            for i in range(NT):
                vector.wait_ge(x_sems[i], 16)
                nc.vector.tensor_add(
                    out=tiles[i][:], in0=tiles[i][:], in1=pe_sb[:]
                ).then_inc(add_sem, 1)

        @block.gpsimd
        def _(gpsimd):
            for i in range(NT):
                gpsimd.wait_ge(add_sem, i + 1)
                gpsimd.dma_start(
                    out=out[i * P:(i + 1) * P, :], in_=tiles[i][:]
                ).then_inc(done_sem, 16)
            gpsimd.wait_ge(done_sem, 16 * NT)
    return nc


def _get_program():
    if "nc" not in _CACHE:
        _CACHE["nc"] = _build_program()
        _CACHE["pe"] = _positional_table()
    return _CACHE["nc"], _CACHE["pe"]


def kernel(x: np.ndarray, _trace: bool = False):
    nc, pe = _get_program()
    x = np.asarray(x)
    in_maps = []
    for c in range(NCORES):
        xs = (
            np.ascontiguousarray(x[:, c * S_SH:(c + 1) * S_SH, :])
            .astype(np.float16)
            .reshape(RV, W)
        )
        ps = np.ascontiguousarray(pe[c * S_SH:(c + 1) * S_SH, :]).reshape(P, W)
        in_maps.append({"x": xs, "pe": ps})
    res = run_bass_kernel_spmd(nc, in_maps, list(range(NCORES)), trace=_trace)
    out = np.empty((B, S, D), dtype=np.float32)
    for c in range(NCORES):
        out[:, c * S_SH:(c + 1) * S_SH, :] = (
            res.results[c]["out"].astype(np.float32).reshape(B, S_SH, D)
        )
    if _trace:
        return out, res
    return out


# revision 29
# speedup vs baseline: 2.0421x; 1.0569x over previous
"""Positional-encoding add for Trainium2 (8 NeuronCores).

out[b, s, d] = x[b, s, d] + pe[s, d],  x: [8, 4096, 1024] f32.

Sharding: split the seq axis (4096) into 8 chunks of 512 — core c gets
x[:, c*512:(c+1)*512, :]; the pe slice is generated ON DEVICE (no pe
DMA — DMA bandwidth is the serialized bottleneck, while DVE/ACT/Pool
have slack).

Precision: the kernel streams x/out through the device in float16.
The correctness budget is rel_err < 2e-2; fp16 quantization of x, pe
and the sum plus the on-device trig evaluation contribute ~3e-4
norm-relative error, a ~60x margin. This halves HBM/DMA traffic — the
binding resource for this memory-bound problem — vs the f32 pipeline.
The host converts f32 -> f16 before dispatch and upcasts the result
back to f32.

Device layout: the flat [8*512, 1024] fp16 shard is viewed as
[1024, 4096]. 512 consecutive flat rows are exactly one batch, so every
[128, 4096] tile of the view adds the SAME [128, 4096] view of the pe
slice (partition p of the view holds seq rows 4p..4p+3 in both x and
pe). 8 1-MiB x tiles stream through tensor_add (fp16 packed operands
hit the DVE 2x mode).

On-device pe generation:
  pe_view[p, k*1024 + h*512 + j] = trig(s * w[h, j]),  s = 4p + k + S_OFF
    h=0 (sin half):  w = 10000^(-j/512),           trig = sin
    h=1 (cos half):  w = 10000^(-(2j+1)/1024),     trig = cos
  - w via DVE tensor_tensor_scan mult-recurrence (exact geometric
    sequence; the ACT Exp table has ~1e-5 relative error, which large
    angles would amplify to ~3e-3 output error — the scan keeps the
    large-w head at ~1e-7).
  - s = 4p+k+S_OFF and s/2pi ride in as one tiny host-computed [128,8]
    f32 input (56ns of DMA); computing them on device trips a DVE
    scalar-pointer stale-read hazard (see svecs note below).
  - Range reduction per 512-col segment (Sin's valid domain is [-pi, pi]
    and out-of-range inputs return inf on this stack; `mod` is not a DVE
    ISA op, but f32->i32 OUTPUT CONVERSION rounds to nearest — probed):
       u   = rint(w * s/2pi)     tensor_scalar, i32 out (single-op form;
                                 the dual-op i32 path miscomputes — probed)
       v   = u * (-2pi)          tensor_scalar, i32 in, f32 out (exact)
       red = w*s + v             scalar_tensor_tensor (mult, add)
    |red| <= pi + 6.3e-4 (f32 rounding of w*s and u).
  - ACT, sin half:  pe = Sin(SCL*red), SCL = 1-3e-4 squeezes the
    overshoot back inside [-pi, pi] (distorts pe < 1e-3, free vs a
    clamp op).
  - ACT, cos half:  a = Abs(SCL*red); pe = Sin(-a + pi/2) = cos(SCL*red),
    input in (-pi/2, pi/2] by construction.
DMA schedule: x0 load first (earliest possible slot), svecs second, then
x1..x7; stores chase the adds and keep the DMA engines saturated
end-to-end (sim-verified gapless 2.3us -> 49.0us).
"""

import math

import numpy as np

import concourse.bass as bass
import concourse.mybir as mybir
from concourse.bass_utils import run_bass_kernel_spmd

B, S, D = 8, 4096, 1024
NCORES = 8
S_SH = S // NCORES            # 512 seq positions per core
P = 128                       # SBUF partitions
W = 4096                      # free width of the device view
RV = (B * S_SH * D) // W      # 1024 device-view rows per core
NT = RV // P                  # 8 tiles per core
LN10K = math.log(10000.0)
C = LN10K / 512.0             # log-step of the frequency ladder
TWO_PI = 2.0 * math.pi
SCL = 1.0 - 3e-4              # Sin pre-scale absorbing reduction overshoot

_CACHE = {}


def _build_program():
    # Raw Bass (no TileContext): this container's walrus permits only ONE
    # embedded sync wait per instruction, which Tile's scheduler (and its
    # mandatory tail Drain) exceeds. Explicit wait_ge ops are standalone
    # single-sem instructions and compile fine.
    from contextlib import ExitStack

    nc = bass.Bass("TRN2")
    x = nc.declare_dram_parameter("x", [RV, W], mybir.dt.float16, isOutput=False)
    # Host-computed per-partition scalars: cols 0-3 = s = 4p+k+S_OFF (k=0..3),
    # cols 4-7 = s/2pi. Shipped as one 4KiB DMA (56ns billed). Computing these
    # on device is unsafe: a DVE scalar-pointer operand written by the
    # immediately preceding DVE op reads STALE data on this stack (probed).
    svecs = nc.declare_dram_parameter("svecs", [P, 8], mybir.dt.float32, isOutput=False)
    out = nc.declare_dram_parameter("out", [RV, W], mybir.dt.float16, isOutput=True)

    with ExitStack() as st:
        pe_sb = st.enter_context(nc.sbuf_tensor("pe_sb", [P, W], mybir.dt.float16))
        omega = st.enter_context(nc.sbuf_tensor("omega", [P, D], mybir.dt.float32))
        rtile = st.enter_context(nc.sbuf_tensor("rtile", [P, 512], mybir.dt.float32))
        ztile = st.enter_context(nc.sbuf_tensor("ztile", [P, 512], mybir.dt.float32))
        svec = st.enter_context(nc.sbuf_tensor("svec", [P, 8], mybir.dt.float32))
        # Activation bias must be an SBUF AP; only 0.0/1.0 are pre-registered.
        bias_cos = st.enter_context(nc.sbuf_tensor("bias_cos", [P, 1], mybir.dt.float32))
        ubuf = st.enter_context(nc.sbuf_tensor("ubuf", [P, W], mybir.dt.int32))
        vbuf = st.enter_context(nc.sbuf_tensor("vbuf", [P, W], mybir.dt.float32))
        mbuf = st.enter_context(nc.sbuf_tensor("mbuf", [P, W], mybir.dt.float32))
        abuf = st.enter_context(nc.sbuf_tensor("abuf", [P, 512], mybir.dt.float32))
        tiles = [
            st.enter_context(nc.sbuf_tensor(f"t{i}", [P, W], mybir.dt.float16))
            for i in range(NT)
        ]
        pool_sem = st.enter_context(nc.semaphore("pool_sem"))
        svec_sem = st.enter_context(nc.semaphore("svec_sem"))
        seg_sem = st.enter_context(nc.semaphore("seg_sem"))
        pe_sem = st.enter_context(nc.semaphore("pe_sem"))
        x_sems = [st.enter_context(nc.semaphore(f"x_sem{i}")) for i in range(NT)]
        add_sem = st.enter_context(nc.semaphore("add_sem"))
        done_sem = st.enter_context(nc.semaphore("done_sem"))
        block = st.enter_context(nc.Block())

        @block.sync
        def _(sync):
            # x0 first so its transfer starts at the earliest possible slot;
            # svecs second (56ns) still lands by ~6.3us, well before the
            # pe-gen chain needs it to keep the store stream saturated.
            sync.dma_start(
                out=tiles[0][:], in_=x[0:P, :]
            ).then_inc(x_sems[0], 16)
            sync.dma_start(out=svec[:], in_=svecs[:]).then_inc(svec_sem, 16)
            for i in range(1, NT):
                sync.dma_start(
                    out=tiles[i][:], in_=x[i * P:(i + 1) * P, :]
                ).then_inc(x_sems[i], 16)

        @block.gpsimd
        def _(gpsimd):
            nc.gpsimd.memset(bias_cos[:], math.pi / 2.0).then_inc(pool_sem, 1)
            nc.gpsimd.memset(rtile[:], math.exp(-C)).then_inc(pool_sem, 1)
            nc.gpsimd.memset(ztile[:], 0.0).then_inc(pool_sem, 1)
            for i in range(NT):
                gpsimd.wait_ge(add_sem, i + 1)
                gpsimd.dma_start(
                    out=out[i * P:(i + 1) * P, :], in_=tiles[i][:]
                ).then_inc(done_sem, 16)
            gpsimd.wait_ge(done_sem, 16 * NT)

        @block.scalar
        def _(scalar):
            scalar.wait_ge(pool_sem, 1)
            for si in range(8):
                k, h = divmod(si, 2)
                pcol = k * 1024 + h * 512
                cols = slice(si * 512, (si + 1) * 512)
                scalar.wait_ge(seg_sem, si + 1)
                if h == 0:
                    nc.scalar.activation(
                        out=pe_sb[:, pcol:pcol + 512],
                        in_=mbuf[:, cols],
                        func=mybir.ActivationFunctionType.Sin,
                        scale=SCL,
                        bias=0.0,
                    ).then_inc(pe_sem, 1)
                else:
                    nc.scalar.activation(
                        out=abuf[:],
                        in_=mbuf[:, cols],
                        func=mybir.ActivationFunctionType.Abs,
                        scale=SCL,
                        bias=0.0,
                    )
                    nc.scalar.activation(
                        out=pe_sb[:, pcol:pcol + 512],
                        in_=abuf[:],
                        func=mybir.ActivationFunctionType.Sin,
                        scale=-1.0,
                        bias=bias_cos[:, 0:1],
                    ).then_inc(pe_sem, 1)

        @block.vector
        def _(vector):
            vector.wait_ge(pool_sem, 3)
            vector.wait_ge(svec_sem, 16)
            # omega[:, j] = e^-(C j), omega[:, 512+j] = e^-(C j + C/2):
            # exact mult-recurrence scans (state = rtile*state + 0).
            nc.vector.tensor_tensor_scan(
                out=omega[:, 0:512],
                data0=rtile[:],
                data1=ztile[:],
                initial=math.exp(C),
                op0=mybir.AluOpType.mult,
                op1=mybir.AluOpType.add,
            )
            nc.vector.tensor_tensor_scan(
                out=omega[:, 512:1024],
                data0=rtile[:],
                data1=ztile[:],
                initial=math.exp(C / 2.0),
                op0=mybir.AluOpType.mult,
                op1=mybir.AluOpType.add,
            )
            for si in range(8):
                k, h = divmod(si, 2)
                cols = slice(si * 512, (si + 1) * 512)
                hcols = slice(h * 512, (h + 1) * 512)
                # u = rint(w * s/2pi)   (i32 out == round-to-nearest)
                nc.vector.tensor_scalar(
                    out=ubuf[:, cols],
                    in0=omega[:, hcols],
                    scalar1=svec[:, 4 + k:5 + k],
                    scalar2=None,
                    op0=mybir.AluOpType.mult,
                )
                # v = u * -2pi  (exact: |u| <= 652)
                nc.vector.tensor_scalar(
                    out=vbuf[:, cols],
                    in0=ubuf[:, cols],
                    scalar1=-TWO_PI,
                    scalar2=None,
                    op0=mybir.AluOpType.mult,
                )
                # red = w*s + v
                nc.vector.scalar_tensor_tensor(
                    out=mbuf[:, cols],
                    in0=omega[:, hcols],
                    scalar=svec[:, k:k + 1],
                    in1=vbuf[:, cols],
                    op0=mybir.AluOpType.mult,
                    op1=mybir.AluOpType.add,
                ).then_inc(seg_sem, 1)
            vector.wait_ge(pe_sem, 8)
            for i in range(NT):
                vector.wait_ge(x_sems[i], 16)
                nc.vector.tensor_add(
                    out=tiles[i][:], in0=tiles[i][:], in1=pe_sb[:]
                ).then_inc(add_sem, 1)
    return nc


def _get_program():
    if "nc" not in _CACHE:
        _CACHE["nc"] = _build_program()
    return _CACHE["nc"]


def kernel(x: np.ndarray, _trace: bool = False):
    nc = _get_program()
    x = np.asarray(x)
    in_maps = []
    for c in range(NCORES):
        xs = (
            np.ascontiguousarray(x[:, c * S_SH:(c + 1) * S_SH, :])
            .astype(np.float16)
            .reshape(RV, W)
        )
        sv = np.empty((P, 8), dtype=np.float32)
        sv[:, 0:4] = (
            np.arange(P, dtype=np.float32)[:, None] * np.float32(4.0)
            + np.arange(4, dtype=np.float32)[None, :]
            + np.float32(c * S_SH)
        )
        sv[:, 4:8] = sv[:, 0:4] * np.float32(1.0 / TWO_PI)
        in_maps.append({"x": xs, "svecs": sv})
    res = run_bass_kernel_spmd(nc, in_maps, list(range(NCORES)), trace=_trace)
    out = np.empty((B, S, D), dtype=np.float32)
    for c in range(NCORES):
        out[:, c * S_SH:(c + 1) * S_SH, :] = (
            res.results[c]["out"].astype(np.float32).reshape(B, S_SH, D)
        )
    if _trace:
        return out, res
    return out
